# revision 15
# baseline (speedup 1.0000x reference)
"""Dynamic per-pixel 3x3 filtering on 8 Trainium2 NeuronCores.

out[b,c,y,x] = sum_{ki,kj} img[b,c,y+ki-1,x+kj-1] * kernels[b,c,ki*3+kj,y,x]
(zero padding outside the image).

Sharding: pure data parallel, one batch sample per core (B=8, 8 cores).

Per-core layout: partition p holds 4 CONSECUTIVE image rows 4p..4p+3
(8 KB contiguous per partition -> single-descriptor DMAs). A +-1 row
shift is then a FREE-DIM shift inside an extended tile
ext[p, bb, xx] = img[4p+bb-1, xx-1] (6 rows x 514 cols, zero padded).
The 6-row window is loaded as ONE overlapping-window DMA (12 KB
contiguous per partition, built with a raw AP since rearrange cannot
express overlap). Re-reading 2 of 6 rows costs +0.5 MB/channel of HBM,
which is cheaper than the latency of producing boundary rows on-chip:
the DVE chain is the end-to-end critical path, and a TensorE
shifted-identity matmul + PSUM evacuation delayed its start by ~13 us.

All elementwise work runs on DVE in fp16: TensorTensor's 2x_1p perf
mode needs every operand to be a packed 2-byte dtype and doubles
throughput vs f32 (measured 2287 -> ~1220 ns per [128,2048] pass).
fp16 chain accumulation keeps max rel err ~1.3e-3, well under the 2e-2
gate. ACT does the f32->fp16 casts of the streamed kernel taps.

Kernel taps stream strictly sequentially from HBM (reordering taps
measurably drops SDMA/HBM efficiency) as nine 1 MB single-descriptor-
per-partition DMAs per channel on the SP HWDGE ring; img windows go on
the ACT HWDGE ring; full-channel stores on the gpsimd SWDGE ring.
Output is stored fp16 (halves store traffic), widened on host.

Tail: the last channel's final tap runs as four cast-free quarter
chains - mixed f32*fp16 mults read the staged tap directly (legal,
runs at 1x) so the post-last-DMA drain has no ACT hop - with quarter
stores on the ACT HWDGE ring (a store's semaphore wait on the load
ring would block subsequent load issues; HWDGE rings are FIFO per
issuing engine).
"""

from contextlib import ExitStack

import numpy as np

import concourse.bacc as bacc
import concourse.mybir as mybir
import concourse.tile as tile
from concourse.ap import AP
from concourse.bass_utils import run_bass_kernel_spmd

C, H, W = 3, 512, 512
KK = 9
NCORES = 8
P = 128
RPB = H // P         # 4 rows per partition
FW = RPB * W         # 2048 free-dim elems of a channel tile
EXT_W = W + 2        # 514: row length incl. zero pad cols
F32 = mybir.dt.float32
F16 = mybir.dt.float16


def _r3(ap):
    """[128, n*W] -> [128, n, W] row-block view."""
    return ap.rearrange("p (b x) -> p b x", x=W)


def _emit(nc, tc, ctx):
    img = nc.dram_tensor("img", (C, H, W), F32, kind="ExternalInput").ap()
    ker = nc.dram_tensor("kernels", (C, KK, H, W), F32, kind="ExternalInput").ap()
    out = nc.dram_tensor("out", (C, H, W), F16, kind="ExternalOutput").ap()

    s_pool = ctx.enter_context(tc.tile_pool(name="imgstage", bufs=2))
    e_pool = ctx.enter_context(tc.tile_pool(name="ext", bufs=2))
    kst_pool = ctx.enter_context(tc.tile_pool(name="kstage", bufs=6))
    kt_pool = ctx.enter_context(tc.tile_pool(name="kt", bufs=12))
    acc_pool = ctx.enter_context(tc.tile_pool(name="acc", bufs=2))
    tmp_pool = ctx.enter_context(tc.tile_pool(name="tmp", bufs=3))

    for c in range(C):
        # --- image window: S[p, j, x] = img[c, 4p-1+j, x]  (f32) ---
        S = s_pool.tile([P, 6, W], F32, tag="S")
        # Out-of-image rows must read as zero; compute-engine accesses must
        # start on an aligned partition, so zero the full row-slots and let
        # the DMAs overwrite every valid partition.
        nc.gpsimd.memset(S[:, 0, :], 0.0)
        nc.gpsimd.memset(S[:, 5, :], 0.0)
        base = img[c]
        win = AP(base.tensor, c * H * W + 3 * W, [[RPB * W, 126], [W, 6], [1, W]])
        nc.scalar.dma_start(S[1:127, :, :], win)
        nc.scalar.dma_start(
            S[0:1, 1:6, :], AP(base.tensor, c * H * W, [[W, 1], [W, 5], [1, W]])
        )
        nc.scalar.dma_start(
            S[127:128, 0:5, :],
            AP(base.tensor, c * H * W + 507 * W, [[W, 1], [W, 5], [1, W]]),
        )

        # --- ext: fp16 [128, 6, 514], zero pad cols, one cast ---
        ext = e_pool.tile([P, 6, EXT_W], F16, tag="ext")
        nc.gpsimd.memset(ext[:, :, 0:1], 0.0)
        nc.gpsimd.memset(ext[:, :, EXT_W - 1 : EXT_W], 0.0)
        nc.scalar.copy(ext[:, :, 1 : W + 1], S[:, :, :])

        # --- kernel taps: stream sequentially, cast f32 -> fp16 ---
        kall = ker[c].rearrange("t (p b) x -> p t (b x)", b=RPB)
        last = c == C - 1

        acc = acc_pool.tile([P, FW], F16, tag="acc")
        out_c = out[c].rearrange("(p b) x -> p (b x)", b=RPB)
        ntap = KK - 1 if last else KK
        for t in range(ntap):
            ki, kj = divmod(t, 3)
            kst = kst_pool.tile([P, FW], F32, tag="kst")
            nc.sync.dma_start(kst[:, :], kall[:, t, :])
            kt = kt_pool.tile([P, FW], F16, tag="kt")
            nc.scalar.copy(kt[:, :], kst[:, :])
            v = ext[:, ki : ki + RPB, kj : kj + W]
            ktap = _r3(kt[:, :])
            if t == 0:
                nc.vector.tensor_mul(_r3(acc[:, :]), v, ktap)
            else:
                tmp = tmp_pool.tile([P, FW], F16, tag="tmp")
                nc.vector.tensor_mul(_r3(tmp[:, :]), v, ktap)
                nc.vector.tensor_add(acc[:, :], acc[:, :], tmp[:, :])
        if not last:
            nc.gpsimd.dma_start(out_c, acc[:, :])
            continue
        # Last tap of the last channel: cast-free quarter chains + quarter
        # stores so the post-last-DMA drain runs at quarter tile size.
        t, ki, kj = KK - 1, 2, 2
        kqs = []
        for q in range(RPB):
            qsl = slice(q * W, (q + 1) * W)
            kq = kst_pool.tile([P, W], F32, tag="kstq")
            nc.sync.dma_start(kq[:, :], kall[:, t, qsl])
            kqs.append(kq)
        for q in range(RPB):
            qsl = slice(q * W, (q + 1) * W)
            tmq = tmp_pool.tile([P, W], F16, tag="tmpq")
            nc.vector.tensor_mul(tmq[:, :], ext[:, ki + q, kj : kj + W], kqs[q][:, :])
            nc.vector.tensor_add(acc[:, qsl], acc[:, qsl], tmq[:, :])
            nc.scalar.dma_start(out_c[:, qsl], acc[:, qsl])


_NC_CACHE = []


def _build():
    nc = bacc.Bacc(
        "TRN2",
        target_bir_lowering=False,
        debug=False,
        enable_asserts=True,
        num_devices=1,
    )
    with tile.TileContext(nc) as tc:
        with ExitStack() as ctx:
            _emit(nc, tc, ctx)
    nc.compile()
    return nc


def kernel(img, kernels):
    """img: [8, 3, 512, 512] f32; kernels: [8, 3, 9, 512, 512] f32.
    Returns [8, 3, 512, 512] f32."""
    first_call = not _NC_CACHE
    if first_call:
        _NC_CACHE.append(_build())
    nc = _NC_CACHE[0]
    img = np.asarray(img, dtype=np.float32)
    kernels = np.asarray(kernels, dtype=np.float32)
    in_maps = [
        {
            "img": np.ascontiguousarray(img[b]),
            "kernels": np.ascontiguousarray(kernels[b]),
        }
        for b in range(NCORES)
    ]
    if first_call:
        # Warm-up execution: the very first run after a fresh NEFF
        # compile/load was observed to occasionally return stale output.
        run_bass_kernel_spmd(nc, in_maps, core_ids=list(range(NCORES)))
    res = run_bass_kernel_spmd(nc, in_maps, core_ids=list(range(NCORES)))
    return np.stack(
        [np.asarray(res.results[b]["out"], dtype=np.float32) for b in range(NCORES)],
        axis=0,
    )
